# revision 1
# baseline (speedup 1.0000x reference)
"""Causal self-attention (B=16, T=1024, C=768, NH=12) on 8 trn2 NeuronCores.

Strategy: data-parallel over the batch dim (2 batches per core, no
collectives). Per batch, projections are computed in "transposed" layouts so
that the softmax reduction lands on the PSUM free dimension:

  xT   [C, T]       via PE transpose of DMA'd x tiles
  qT   [768, T]     = W_q-chunk-lhsT @ xT  (2 heads per 128-row tile)
  kT   [768, T]     likewise
  v    [T, 768]     = xT-chunk-lhsT @ W_v  (natural layout, + a ones column
                      per head so attn@v also yields the softmax denominator
                      Z in psum row 64)
  scoresT[k, q]     = kT-slice-lhsT @ qT-slice  (K=64; the two heads of a
                      pair run concurrently in array rows 0-63 / 64-127)
  attnT             = exp(scoresT / 8) on ACT, diagonal tiles masked on DVE
  avT [65, q]       = v_aug-lhsT @ attnT, accumulated over k chunks in PSUM
  attOutT[d, q]     = avT[0:64] * (1/Z broadcast via K=1 matmul)
  y    [T, C]       = attOutT-chunk-lhsT @ W_proj + bias

All matmuls run in float32r (TF32-like rounding) at full PE rate.
"""
import numpy as np

B, T, C = 16, 1024, 768
NH, HD = 12, 64
NCORES = 8
BPC = B // NCORES          # batches per core
NP = NH // 2               # head pairs (128-row o-tiles)
NT = T // 128              # 128-row seq tiles
NST = T // 512             # 512-col q supertiles
NKC = C // 128             # 128-row contraction chunks of C

_CACHE = {}


def _score_cols(st, kc):
    """Column layout for the (st, kc) score tile within its supertile.

    Returns (start, diag, wide): start = first computed q column (widened so
    N >= 256 keeps fp32r at full rate); diag = needs causal masking;
    wide = mask covers 256 cols (off == 384 case).
    """
    off = 128 * kc - 512 * st
    if off < 0:
        return 0, False, False
    if off == 384:
        return 256, True, True
    return off, True, False


def _build_nc():
    import concourse.bacc as bacc
    import concourse.mybir as mybir
    import concourse.tile as tile

    F32 = mybir.dt.float32
    F32R = mybir.dt.float32r
    EXP = mybir.ActivationFunctionType.Exp

    nc = bacc.Bacc("TRN2", target_bir_lowering=False)

    x_in = nc.dram_tensor("x", [BPC, T, C], F32, kind="ExternalInput")
    wa = nc.dram_tensor("wa", [C, 3 * C], F32, kind="ExternalInput")
    wp = nc.dram_tensor("wp", [C, C], F32, kind="ExternalInput")
    bqk = nc.dram_tensor("bqk", [128, 2 * NP], F32, kind="ExternalInput")
    bv = nc.dram_tensor("bv", [128, C], F32, kind="ExternalInput")
    bp = nc.dram_tensor("bp", [128, C], F32, kind="ExternalInput")
    mask = nc.dram_tensor("mask", [128, 256], F32, kind="ExternalInput")
    ident = nc.dram_tensor("ident", [128, 128], F32, kind="ExternalInput")
    ones = nc.dram_tensor("ones", [128, 64], F32, kind="ExternalInput")
    y_out = nc.dram_tensor("y", [BPC, T, C], F32, kind="ExternalOutput")

    with tile.TileContext(nc) as tc:
        with (
            tc.tile_pool(name="consts", bufs=1) as consts,
            tc.tile_pool(name="sb", bufs=1) as sb,
            tc.tile_pool(name="ps", bufs=1, space="PSUM") as ps,
        ):
            # ---- resident weights / constants -------------------------
            wa_t = []
            for kc in range(NKC):
                t = consts.tile([128, 3 * C], F32R, tag=f"wa{kc}")
                nc.gpsimd.dma_start(t[:], wa[128 * kc:128 * kc + 128, :])
                wa_t.append(t)
            wp_t = []
            for hp in range(NP):
                t = consts.tile([128, C], F32R, tag=f"wp{hp}")
                nc.gpsimd.dma_start(t[:], wp[128 * hp:128 * hp + 128, :])
                wp_t.append(t)
            bqk_sb = consts.tile([128, 2 * NP], F32, tag="bqk")
            nc.gpsimd.dma_start(bqk_sb[:], bqk[:])
            bv_sb = consts.tile([128, C], F32, tag="bv")
            nc.gpsimd.dma_start(bv_sb[:], bv[:])
            bp_sb = consts.tile([128, C], F32, tag="bp")
            nc.gpsimd.dma_start(bp_sb[:], bp[:])
            mask_sb = consts.tile([128, 256], F32R, tag="mask")
            nc.gpsimd.dma_start(mask_sb[:], mask[:])
            ident_sb = consts.tile([128, 128], F32, tag="ident")
            nc.gpsimd.dma_start(ident_sb[:], ident[:])
            ones_sb = consts.tile([128, 64], F32R, tag="ones")
            nc.gpsimd.dma_start(ones_sb[:], ones[:])

            for b in range(BPC):
                # ---- x load + transpose to xT chunks ------------------
                # "big" tag is shared by xT (QKV phase) and attOutT
                # (attention/proj phase) to halve peak SBUF.
                xT = [sb.tile([128, T], F32R, tag="xT", bufs=6, name=f"xT{b}_{i}")
                      for i in range(NKC)]
                for tr in range(NT):
                    x_t = sb.tile([128, C], F32, tag="xin", bufs=2)
                    nc.gpsimd.dma_start(
                        x_t[:], x_in[b, 128 * tr:128 * tr + 128, :])
                    for tc_ in range(NKC):
                        tp_ps = ps.tile([128, 128], F32, tag="tp", bufs=1)
                        nc.tensor.transpose(
                            tp_ps[:], x_t[:, 128 * tc_:128 * tc_ + 128],
                            ident_sb[:])
                        nc.vector.tensor_copy(
                            xT[tc_][:, 128 * tr:128 * tr + 128], tp_ps[:])

                # ---- v projection into v_aug [128, 12*65] -------------
                v_aug = [sb.tile([128, NH * 65], F32R, tag="vaug", bufs=8, name=f"vaug{b}_{i}")
                         for i in range(NT)]
                for s in range(2):
                    for tt in range(NT):
                        pv = ps.tile([128, 384], F32, tag="qkvp", bufs=2)
                        for kc in range(NKC):
                            nc.tensor.matmul(
                                pv[:],
                                xT[kc][:, 128 * tt:128 * tt + 128],
                                wa_t[kc][:, 2 * C + 384 * s:
                                         2 * C + 384 * s + 384],
                                start=(kc == 0), stop=(kc == NKC - 1))
                        va_v = v_aug[tt][:].rearrange(
                            "p (h c) -> p h c", c=65)[:, 6 * s:6 * s + 6, :]
                        nc.vector.tensor_add(
                            out=va_v[:, :, 0:64],
                            in0=pv[:].rearrange("p (h d) -> p h d", d=64),
                            in1=bv_sb[:, 384 * s:384 * s + 384].rearrange(
                                "p (h d) -> p h d", d=64))
                        nc.vector.tensor_copy(va_v[:, :, 64], ones_sb[:, 0:6])

                # ---- per head-pair: qT/kT projection + attention ------
                attOutT = []
                for hp in range(NP):
                    qT = sb.tile([128, T], F32R, tag="qT", bufs=2)
                    kT = sb.tile([128, T], F32R, tag="kT", bufs=2)
                    for dst, osel, bcol in ((qT, 0, hp), (kT, 1, NP + hp)):
                        obase = C * osel + 128 * hp
                        for st in range(NST):
                            pq = ps.tile([128, 512], F32, tag="qkvp", bufs=2)
                            for kc in range(NKC):
                                nc.tensor.matmul(
                                    pq[:],
                                    wa_t[kc][:, obase:obase + 128],
                                    xT[kc][:, 512 * st:512 * st + 512],
                                    start=(kc == 0), stop=(kc == NKC - 1))
                            nc.vector.tensor_scalar_add(
                                out=dst[:, 512 * st:512 * st + 512],
                                in0=pq[:],
                                scalar1=bqk_sb[:, bcol:bcol + 1])

                    aot = sb.tile([128, T], F32R, tag="aot", bufs=6)
                    attOutT.append(aot)
                    for st in range(NST):
                        nkc_av = 4 * (st + 1)
                        av = [ps.tile([65, 512], F32, tag="avp", bufs=3, name=f"av{i}")
                              for i in range(2)]
                        for kc in range(nkc_av):
                            start, diag, wide = _score_cols(st, kc)
                            n = 512 - start
                            at_pair = []
                            sc_pair = []
                            for par in range(2):
                                scp = ps.tile([128, 512], F32, tag="scp",
                                              bufs=2)
                                sc_pair.append(scp)
                                nc.tensor.matmul(
                                    scp[:, 0:n],
                                    kT[64 * par:64 * par + 64,
                                       128 * kc:128 * kc + 128],
                                    qT[64 * par:64 * par + 64,
                                       512 * st + start:512 * st + 512],
                                    start=True, stop=True)
                            for par in range(2):
                                at = sb.tile([128, 512], F32R, tag="attnT",
                                             bufs=4)
                                at_pair.append(at)
                                nc.scalar.activation(
                                    at[:, 0:n], sc_pair[par][:, 0:n], EXP,
                                    scale=0.125)
                                if diag:
                                    w = 256 if wide else 128
                                    msrc = (mask_sb[:, 0:256] if wide
                                            else mask_sb[:, 128:256])
                                    nc.vector.tensor_mul(
                                        out=at[:, 0:w], in0=at[:, 0:w],
                                        in1=msrc)
                            for par in range(2):
                                h = 2 * hp + par
                                nc.tensor.matmul(
                                    av[par][0:65, start:512],
                                    v_aug[kc][:, 65 * h:65 * h + 65],
                                    at_pair[par][:, 0:n],
                                    start=(kc == 0), stop=(kc == nkc_av - 1))
                        # normalize: attOut = av[0:64] / Z  (Z = av row 64)
                        for par in range(2):
                            # Z row (psum row 64) -> f32r on ACT, broadcast Z
                            # to 64 rows via K=1 matmul, 1/Z on 64 lanes
                            rc = sb.tile([65, 512], F32R, tag="rcp", bufs=2)
                            nc.scalar.copy(rc[64:65, :], av[par][64:65, :])
                            bc = ps.tile([64, 512], F32, tag="scp", bufs=2)
                            nc.tensor.matmul(
                                bc[:], ones_sb[64:65, 0:64], rc[64:65, :],
                                start=True, stop=True)
                            rcs = sb.tile([64, 512], F32, tag="rcf", bufs=2)
                            nc.vector.reciprocal_approx_fast(
                                out=rcs[:], in_=bc[:])
                            if par == 0:
                                nc.vector.tensor_mul(
                                    out=aot[0:64, 512 * st:512 * st + 512],
                                    in0=av[par][0:64, :], in1=rcs[:])
                            else:
                                st2 = sb.tile([64, 512], F32R, tag="stg2",
                                              bufs=2)
                                nc.vector.tensor_mul(
                                    out=st2[:], in0=av[par][0:64, :],
                                    in1=rcs[:])
                                nc.gpsimd.dma_start(
                                    aot[64:128, 512 * st:512 * st + 512],
                                    st2[:])

                # ---- output projection + bias -------------------------
                for tt in range(NT):
                    y_sb = sb.tile([128, C], F32, tag="ysb", bufs=2)
                    for s in range(2):
                        py = ps.tile([128, 384], F32, tag="qkvp", bufs=2)
                        for hp in range(NP):
                            nc.tensor.matmul(
                                py[:],
                                attOutT[hp][:, 128 * tt:128 * tt + 128],
                                wp_t[hp][:, 384 * s:384 * s + 384],
                                start=(hp == 0), stop=(hp == NP - 1))
                        nc.vector.tensor_add(
                            out=y_sb[:, 384 * s:384 * s + 384],
                            in0=py[:],
                            in1=bp_sb[:, 384 * s:384 * s + 384])
                    nc.gpsimd.dma_start(
                        y_out[b, 128 * tt:128 * tt + 128, :], y_sb[:])

    nc.finalize()
    return nc


def _prep_const_inputs(W_attn, b_attn, W_proj, b_proj):
    bqk = np.ascontiguousarray(
        b_attn[:2 * C].reshape(2 * NP, 128).T).astype(np.float32)
    bv = np.broadcast_to(b_attn[2 * C:], (128, C)).copy().astype(np.float32)
    bp = np.broadcast_to(b_proj, (128, C)).copy().astype(np.float32)
    # mask[i, jj] = 1 if jj - 128 >= i  (cols 128:256 = standard triangle)
    jj = np.arange(256)[None, :]
    ii = np.arange(128)[:, None]
    mask = (jj - 128 >= ii).astype(np.float32)
    ident = np.eye(128, dtype=np.float32)
    ones = np.ones((128, 64), dtype=np.float32)
    return {
        "wa": np.ascontiguousarray(W_attn, dtype=np.float32),
        "wp": np.ascontiguousarray(W_proj, dtype=np.float32),
        "bqk": bqk, "bv": bv, "bp": bp,
        "mask": mask, "ident": ident, "ones": ones,
    }


def kernel(x, W_attn, b_attn, W_proj, b_proj):
    from concourse.bass_utils import run_bass_kernel_spmd

    if "nc" not in _CACHE:
        _CACHE["nc"] = _build_nc()
    nc = _CACHE["nc"]

    consts = _prep_const_inputs(W_attn, b_attn, W_proj, b_proj)
    x = np.ascontiguousarray(x, dtype=np.float32)
    in_maps = [
        {"x": x[BPC * c:BPC * (c + 1)], **consts} for c in range(NCORES)
    ]
    res = run_bass_kernel_spmd(nc, in_maps, list(range(NCORES)))
    return np.concatenate([r["y"] for r in res.results], axis=0)



# revision 8
# speedup vs baseline: 1.8241x; 1.8241x over previous
"""Causal self-attention (B=16, T=1024, C=768, NH=12) on 8 trn2 NeuronCores.

Data-parallel over the batch dim (2 batches per core, no collectives).
All matmuls run in bf16 (fp32 PSUM accumulation); inputs are cast to bf16
and x is pre-transposed on the host. Weights are repacked on the host so
each SBUF-resident tensor loads with large per-partition DMA packets.

Per batch, layouts keep the softmax reduction on the PSUM free dimension:

  xT   [C, T]       host-pretransposed, DMA'd directly
  qT   [768, T]     = W_q-chunk-lhsT @ xT  (2 heads per 128-row tile)
  kT   [768, T]     likewise
  v    [T, 768]     = xT-chunk-lhsT @ W_v  (+ a ones column per head so
                      attn@v also yields the softmax denominator Z)
  scoresT[k, q]     = kT-slice-lhsT @ qT-slice  (K=64; the two heads of a
                      pair run concurrently in array rows 0-63 / 64-127)
  attnT             = exp(scoresT / 8) on ACT, diagonal tiles masked on DVE
  avT [65, q]       = v_aug-lhsT @ attnT, accumulated over k chunks in PSUM
  attOutT[d, q]     = avT[0:64] * (1/Z broadcast via K=1 matmul)
  y    [T, C]       = attOutT-chunk-lhsT @ W_proj + bias   (bf16 out)

The projection work of the *other* batch (v-proj of b+1, y-proj of b-1) is
interleaved into each batch's attention phase so the tensor engine's
activity monitor (HAM) keeps the PE clock at full rate.
"""
import numpy as np

B, T, C = 16, 1024, 768
NH, HD = 12, 64
NCORES = 8
BPC = B // NCORES          # batches per core
NP = NH // 2               # head pairs (128-row o-tiles)
NT = T // 128              # 128-row seq tiles
NST = T // 512             # 512-col q supertiles
NKC = C // 128             # 128-row contraction chunks of C

_CACHE = {}


def _build_nc():
    import concourse.bacc as bacc
    import concourse.mybir as mybir
    import concourse.tile as tile

    F32 = mybir.dt.float32
    BF16 = mybir.dt.bfloat16
    EXP = mybir.ActivationFunctionType.Exp

    nc = bacc.Bacc("TRN2", target_bir_lowering=False)

    # xt rows are C (chunked 128/partition-tile); cols are [b0 T | b1 T]
    xt_in = nc.dram_tensor("xt", [C, BPC * T], BF16, kind="ExternalInput")
    # wa packed [128, NKC*3C]: col block kc holds wa[128*kc:128*kc+128, :]
    wa = nc.dram_tensor("wa", [128, NKC * 3 * C], BF16, kind="ExternalInput")
    # wp packed [128, NP*C]: col block hp holds wp[128*hp:128*hp+128, :]
    wp = nc.dram_tensor("wp", [128, NP * C], BF16, kind="ExternalInput")
    bqk = nc.dram_tensor("bqk", [128, 2 * NP], F32, kind="ExternalInput")
    bv = nc.dram_tensor("bv", [128, C], BF16, kind="ExternalInput")
    bp = nc.dram_tensor("bp", [128, C], BF16, kind="ExternalInput")
    mask = nc.dram_tensor("mask", [128, 128], BF16, kind="ExternalInput")
    ones = nc.dram_tensor("ones", [128, 64], BF16, kind="ExternalInput")
    y_out = nc.dram_tensor("y", [BPC, T, C], BF16, kind="ExternalOutput")

    with tile.TileContext(nc) as tc:
        with (
            tc.tile_pool(name="consts", bufs=1) as consts,
            tc.tile_pool(name="sb", bufs=1) as sb,
            tc.tile_pool(name="ps", bufs=1, space="PSUM") as ps,
        ):
            # ---- resident weights / constants (split across queues,
            # ordered so the first compute's inputs arrive first) --------
            wa_sb = consts.tile([128, NKC * 3 * C], BF16, tag="wa")
            wa_t = [wa_sb[:, 3 * C * kc:3 * C * (kc + 1)] for kc in range(NKC)]

            def wa_block(ap, chunks, lo, hi):
                v = ap[:, 3 * C * chunks.start:3 * C * chunks.stop]
                v = v.rearrange("p (c w) -> p c w", w=3 * C)
                return v[:, :, lo:hi]

            # qk columns of wa first (gate the first qkT chains)
            nc.gpsimd.dma_start(wa_block(wa_sb, slice(0, 3), 0, 2 * C),
                                wa_block(wa, slice(0, 3), 0, 2 * C))
            nc.scalar.dma_start(wa_block(wa_sb, slice(3, 6), 0, 2 * C),
                                wa_block(wa, slice(3, 6), 0, 2 * C))
            # xT: batch-0 columns first (gate everything)
            xT = [sb.tile([128, BPC * T], BF16, tag="xT", bufs=NKC,
                          name=f"xT{c}") for c in range(NKC)]
            for b in range(BPC):
                for c in range(NKC):
                    nc.sync.dma_start(
                        xT[c][:, T * b:T * b + T],
                        xt_in[128 * c:128 * c + 128, T * b:T * b + T])
            # v columns of wa next
            nc.gpsimd.dma_start(wa_block(wa_sb, slice(0, 3), 2 * C, 3 * C),
                                wa_block(wa, slice(0, 3), 2 * C, 3 * C))
            nc.scalar.dma_start(wa_block(wa_sb, slice(3, 6), 2 * C, 3 * C),
                                wa_block(wa, slice(3, 6), 2 * C, 3 * C))

            bqk_sb = consts.tile([128, 2 * NP], F32, tag="bqk")
            nc.scalar.dma_start(bqk_sb[:], bqk[:])
            bv_sb = consts.tile([128, C], BF16, tag="bv")
            nc.scalar.dma_start(bv_sb[:], bv[:])
            mask_sb = consts.tile([128, 128], BF16, tag="mask")
            nc.scalar.dma_start(mask_sb[:], mask[:])
            ones_sb = consts.tile([128, 64], BF16, tag="ones")
            nc.scalar.dma_start(ones_sb[:], ones[:])

            wp_sb = consts.tile([128, NP * C], BF16, tag="wp")
            nc.gpsimd.dma_start(wp_sb[:], wp[:])
            wp_t = [wp_sb[:, C * hp:C * (hp + 1)] for hp in range(NP)]
            bp_sb = consts.tile([128, C], BF16, tag="bp")
            nc.gpsimd.dma_start(bp_sb[:], bp[:])

            def x_of(b, kc, lo, hi):
                return xT[kc][:, T * b + lo:T * b + hi]

            def emit_qkT(b, hp):
                qT = sb.tile([128, T], BF16, tag="qT", bufs=3)
                kT = sb.tile([128, T], BF16, tag="kT", bufs=3)
                for dst, osel, bcol in ((qT, 0, hp), (kT, 1, NP + hp)):
                    obase = C * osel + 128 * hp
                    for st in range(NST):
                        pq = ps.tile([128, 512], F32, tag="pp", bufs=2)
                        for kc in range(NKC):
                            nc.tensor.matmul(
                                pq[:],
                                wa_t[kc][:, obase:obase + 128],
                                x_of(b, kc, 512 * st, 512 * st + 512),
                                start=(kc == 0), stop=(kc == NKC - 1))
                        nc.vector.tensor_scalar_add(
                            out=dst[:, 512 * st:512 * st + 512],
                            in0=pq[:],
                            scalar1=bqk_sb[:, bcol:bcol + 1])
                return qT, kT

            v_aug = {}

            def emit_vproj_chain(b, s, tt):
                pv = ps.tile([128, 512], F32, tag="pp", bufs=2)
                for kc in range(NKC):
                    nc.tensor.matmul(
                        pv[:, 0:384],
                        x_of(b, kc, 128 * tt, 128 * tt + 128),
                        wa_t[kc][:, 2 * C + 384 * s:2 * C + 384 * s + 384],
                        start=(kc == 0), stop=(kc == NKC - 1))
                va_v = v_aug[b][tt][:].rearrange(
                    "p (h c) -> p h c", c=65)[:, 6 * s:6 * s + 6, :]
                nc.vector.tensor_add(
                    out=va_v[:, :, 0:64],
                    in0=pv[:, 0:384].rearrange("p (h d) -> p h d", d=64),
                    in1=bv_sb[:, 384 * s:384 * s + 384].rearrange(
                        "p (h d) -> p h d", d=64))
                nc.vector.tensor_copy(va_v[:, :, 64], ones_sb[:, 0:6])

            def alloc_vaug(b):
                v_aug[b] = [sb.tile([128, NH * 65], BF16, tag="vaug",
                                    bufs=BPC * NT, name=f"vaug{b}_{i}")
                            for i in range(NT)]

            attOutT = {}

            def emit_yproj_tt(b, tt):
                y_sb = sb.tile([128, C], BF16, tag="ysb", bufs=3)
                for s in range(2):
                    py = ps.tile([128, 512], F32, tag="pp", bufs=2)
                    for hp in range(NP):
                        nc.tensor.matmul(
                            py[:, 0:384],
                            attOutT[b][hp][:, 128 * tt:128 * tt + 128],
                            wp_t[hp][:, 384 * s:384 * s + 384],
                            start=(hp == 0), stop=(hp == NP - 1))
                    nc.vector.tensor_add(
                        out=y_sb[:, 384 * s:384 * s + 384],
                        in0=py[:, 0:384],
                        in1=bp_sb[:, 384 * s:384 * s + 384])
                eng = (nc.sync, nc.gpsimd, nc.scalar)[tt % 3]
                eng.dma_start(y_out[b, 128 * tt:128 * tt + 128, :], y_sb[:])

            def attn_st(b, hp, st, qT, kT, aot, st2):
                nkc_av = 4 * (st + 1)
                av = [ps.tile([65, 512], F32, tag="avp", bufs=2,
                              name=f"av{i}") for i in range(2)]
                for kc in range(nkc_av):
                    off = 128 * kc - 512 * st
                    diag = off >= 0
                    start = max(off, 0)
                    n = 512 - start
                    sc_pair = []
                    at_pair = []
                    for par in range(2):
                        scp = ps.tile([128, 512], F32, tag="scp", bufs=4)
                        sc_pair.append(scp)
                        nc.tensor.matmul(
                            scp[:, 0:n],
                            kT[64 * par:64 * par + 64,
                               128 * kc:128 * kc + 128],
                            qT[64 * par:64 * par + 64,
                               512 * st + start:512 * st + 512],
                            start=True, stop=True)
                    for par in range(2):
                        at = sb.tile([128, 512], BF16, tag="attnT", bufs=6)
                        at_pair.append(at)
                        nc.scalar.activation(
                            at[:, 0:n], sc_pair[par][:, 0:n], EXP,
                            scale=0.125)
                        if diag:
                            nc.vector.tensor_mul(
                                out=at[:, 0:128], in0=at[:, 0:128],
                                in1=mask_sb[:])
                    for par in range(2):
                        h = 2 * hp + par
                        nc.tensor.matmul(
                            av[par][0:65, start:512],
                            v_aug[b][kc][:, 65 * h:65 * h + 65],
                            at_pair[par][:, 0:n],
                            start=(kc == 0), stop=(kc == nkc_av - 1))
                # normalize: attOut = av[0:64] / Z  (Z = av row 64)
                # phase-wise across the pair so the chain pipelines
                rc_pair = []
                for par in range(2):
                    rc = sb.tile([128, 512], BF16, tag="rcp", bufs=2)
                    nc.scalar.copy(rc[64:65, :], av[par][64:65, :])
                    rc_pair.append(rc)
                bc_pair = []
                for par in range(2):
                    bcp = ps.tile([128, 512], F32, tag="scp", bufs=4)
                    nc.tensor.matmul(
                        bcp[0:64, :], ones_sb[64:65, 0:64],
                        rc_pair[par][64:65, :], start=True, stop=True)
                    bc_pair.append(bcp)
                rr_pair = []
                for par in range(2):
                    rcs = sb.tile([64, 512], F32, tag="rcf", bufs=2)
                    nc.vector.reciprocal_approx_fast(
                        out=rcs[:], in_=bc_pair[par][0:64, :])
                    rr_pair.append(rcs)
                nc.vector.tensor_mul(
                    out=aot[0:64, 512 * st:512 * st + 512],
                    in0=av[0][0:64, :], in1=rr_pair[0][:])
                nc.vector.tensor_mul(
                    out=st2[:, 512 * st:512 * st + 512],
                    in0=av[1][0:64, :], in1=rr_pair[1][:])

            # ================= main schedule ==========================
            alloc_vaug(0)
            for b in range(BPC):
                qk = [emit_qkT(b, 0)]
                if b == 0:
                    # first batch's v-proj runs up front (dense warm-up)
                    for s in range(2):
                        for tt in range(NT):
                            emit_vproj_chain(0, s, tt)
                    alloc_vaug(1)
                    filler = [(emit_vproj_chain, (1, s, tt))
                              for s in range(2) for tt in range(NT)]
                else:
                    filler = [(emit_yproj_tt, (0, tt)) for tt in range(NT)]
                fi = 0

                attOutT[b] = []
                for hp in range(NP):
                    if hp + 1 < NP:
                        qk.append(emit_qkT(b, hp + 1))
                    qT, kT = qk[hp]
                    aot = sb.tile([128, T], BF16, tag="aot", bufs=2 * NP + 1)
                    attOutT[b].append(aot)
                    st2 = sb.tile([64, T], BF16, tag="stg2", bufs=2)

                    attn_st(b, hp, 0, qT, kT, aot, st2)
                    # interleave dense projection work of the other batch
                    n_fill = (len(filler) * (hp + 1) * 2 + NP * 2 - 1) \
                        // (NP * 2)
                    while fi < min(n_fill, len(filler)):
                        f, a = filler[fi]
                        f(*a)
                        fi += 1
                    attn_st(b, hp, 1, qT, kT, aot, st2)
                    n_fill = (len(filler) * ((hp + 1) * 2 + 1)
                              + NP * 2 - 1) // (NP * 2)
                    while fi < min(n_fill, len(filler)):
                        f, a = filler[fi]
                        f(*a)
                        fi += 1
                    # par=1 halves -> aot rows 64:128 (one DMA per hp)
                    nc.sync.dma_start(aot[64:128, :], st2[:])
                while fi < len(filler):
                    f, a = filler[fi]
                    f(*a)
                    fi += 1
            # final batch's y projection (dense tail)
            for tt in range(NT):
                emit_yproj_tt(1, tt)

    nc.finalize()
    return nc


def _prep_const_inputs(W_attn, b_attn, W_proj, b_proj):
    import ml_dtypes
    BF = ml_dtypes.bfloat16
    W_attn = np.asarray(W_attn, dtype=np.float32)
    W_proj = np.asarray(W_proj, dtype=np.float32)
    b_attn = np.asarray(b_attn, dtype=np.float32)
    b_proj = np.asarray(b_proj, dtype=np.float32)
    # wa packed: col block kc = wa[128*kc:128*(kc+1), :]  -> [128, NKC*3C]
    wa = np.ascontiguousarray(
        W_attn.reshape(NKC, 128, 3 * C).transpose(1, 0, 2).reshape(
            128, NKC * 3 * C)).astype(BF)
    wp = np.ascontiguousarray(
        W_proj.reshape(NP, 128, C).transpose(1, 0, 2).reshape(
            128, NP * C)).astype(BF)
    bqk = np.ascontiguousarray(
        b_attn[:2 * C].reshape(2 * NP, 128).T).astype(np.float32)
    bv = np.ascontiguousarray(
        np.broadcast_to(b_attn[2 * C:], (128, C))).astype(BF)
    bp = np.ascontiguousarray(
        np.broadcast_to(b_proj, (128, C))).astype(BF)
    # diag-tile mask: mask[i, j] = 1 if j >= i  (q-col >= k-row)
    jj = np.arange(128)[None, :]
    ii = np.arange(128)[:, None]
    mask = (jj >= ii).astype(BF)
    ones = np.ones((128, 64), dtype=BF)
    return {"wa": wa, "wp": wp, "bqk": bqk, "bv": bv, "bp": bp,
            "mask": mask, "ones": ones}


def _make_in_maps(x, W_attn, b_attn, W_proj, b_proj):
    import ml_dtypes
    BF = ml_dtypes.bfloat16
    consts = _prep_const_inputs(W_attn, b_attn, W_proj, b_proj)
    xb = np.asarray(x, dtype=np.float32).astype(BF)
    maps = []
    for c in range(NCORES):
        xc = xb[BPC * c:BPC * (c + 1)]            # [BPC, T, C]
        xtc = np.ascontiguousarray(
            xc.transpose(2, 0, 1).reshape(C, BPC * T))
        maps.append({"xt": xtc, **consts})
    return maps


def kernel(x, W_attn, b_attn, W_proj, b_proj):
    from concourse.bass_utils import run_bass_kernel_spmd

    if "nc" not in _CACHE:
        _CACHE["nc"] = _build_nc()
    nc = _CACHE["nc"]

    in_maps = _make_in_maps(x, W_attn, b_attn, W_proj, b_proj)
    res = run_bass_kernel_spmd(nc, in_maps, list(range(NCORES)))
    return np.concatenate(
        [np.asarray(r["y"], dtype=np.float32) for r in res.results], axis=0)
